# revision 1
# baseline (speedup 1.0000x reference)
"""Trainium2 Bass kernel for GQA MHA prefill (S=2048, D=4096, H=32, KVH=8).

Strategy (8 NeuronCores, tensor-parallel over heads):
  - Each core owns 4 query heads + 1 kv head. Host stages transposed,
    head-permuted weight shards so no on-chip transposes are needed for
    the projections: qT/kT/vT come out of the PE directly in [dim, seq]
    layout (seq on the free axis).
  - Head-dim components are permuted (even indices first, odd second) so
    RoPE becomes ops on contiguous partition halves; the permutation is
    applied identically to q and k so logits are unchanged.
  - SDPA runs in transposed layout: logitsT[k, q] = kT.T @ qT per
    128-row k-chunk; exp on ScalarE (no max subtraction needed: logits
    are O(5) by construction); causal masking is structural
    (skip upper blocks + affine_select on diagonal tiles). Softmax
    denominators come free as an extra ones-column in the p@v matmul.
  - o[q, hd] tiles are normalized, PE-transposed to oT and AllToAll'd
    (4 x 1MB per core) from head-sharded to seq-sharded layout, then each
    core computes its 256 output rows against the full wo (streamed).
  - If the mask input is NOT the expected causal mask, a general
    fallback variant applies the mask as data (identity-matmul
    accumulation into PSUM).
"""

import sys

import numpy as np

sys.path.insert(0, "/opt/trn_rl_repo")

S = 2048
D = 4096
H = 32
KVH = 8
HD = 128
NCORES = 8
HL = H // NCORES          # 4 local query heads
DL = HL * HD              # 512 local q dim
SQ = S // NCORES          # 256 output rows per core
GH = HD // 2              # 64 rope pair lanes
KC = S // 128             # 16 key chunks
DC = D // 128             # 32 contraction chunks
NB = S // 512             # 4 seq blocks of 512
QB = S // 512             # 4 q blocks of 512
NEG = -1e9
VST = 130                 # v_sb column stride: 128 hd + 1 ones + 1 pad

_built = {}


def _build(causal: bool, for_sim: bool = False):
    import concourse.bass as bass  # noqa: F401
    import concourse.mybir as mybir
    import concourse.tile as tile
    from concourse import bacc
    from concourse.masks import make_identity

    fp32 = mybir.dt.float32
    bf16 = mybir.dt.bfloat16
    AF = mybir.ActivationFunctionType
    OP = mybir.AluOpType

    nc = bacc.Bacc(
        "TRN2",
        target_bir_lowering=False,
        debug=False,
        num_devices=1 if for_sim else NCORES,
    )
    f32r = mybir.dt.float32r
    xT = nc.dram_tensor("xT", [DC, NB, 128, 512], f32r, kind="ExternalInput")
    wqT = nc.dram_tensor("wqT", [D, DL], f32r, kind="ExternalInput")
    wkT = nc.dram_tensor("wkT", [D, HD], f32r, kind="ExternalInput")
    wvT = nc.dram_tensor("wvT", [D, HD], f32r, kind="ExternalInput")
    cosT = nc.dram_tensor("cosT", [128, S], fp32, kind="ExternalInput")
    sinT = nc.dram_tensor("sinT", [128, S], fp32, kind="ExternalInput")
    woT = nc.dram_tensor("woT", [DC, 4, 128, 1024], bf16, kind="ExternalInput")
    if not causal:
        maskT = nc.dram_tensor("maskT", [S, S], fp32, kind="ExternalInput")
    out = nc.dram_tensor("out", [SQ, D], fp32, kind="ExternalOutput")

    rg = [list(range(NCORES))]

    with tile.TileContext(nc) as tc:
        with (
            tc.tile_pool(name="const", bufs=1) as constp,
            tc.tile_pool(name="pers", bufs=1) as pers,
            tc.tile_pool(name="dram", bufs=1, space="DRAM") as dramp,
        ):
            ident = constp.tile([128, 128], fp32, tag="ident")
            make_identity(nc, ident[:])
            c_sb = constp.tile([128, S], fp32, tag="cos")
            s_sb = constp.tile([128, S], fp32, tag="sin")
            # half-swap permutation: (Psw^T x)[p] = x[(p+64) % 128]
            psw = constp.tile([128, 128], fp32, tag="psw")
            nc.gpsimd.memset(psw[:], 0.0)
            for b0 in (64, -64):
                nc.gpsimd.affine_select(
                    out=psw[:], in_=psw[:],
                    pattern=[[-1, 128]],
                    compare_op=OP.not_equal,
                    fill=1.0,
                    base=b0,
                    channel_multiplier=1,
                )
            pswr = constp.tile([128, 128], f32r, tag="pswr")
            nc.scalar.copy(pswr[:], psw[:])

            qT_sb = pers.tile([128, HL * S], f32r, tag="qT")
            kT_sb = pers.tile([128, S], f32r, tag="kT")
            v_sb = pers.tile([128, KC * VST], bf16, tag="v")

            a2a_in = [
                dramp.tile(
                    [NCORES * HD, SQ], bf16,
                    tag=f"a2a_in{h}", name=f"a2a_in{h}",
                )
                for h in range(HL)
            ]
            a2a_out = [
                dramp.tile(
                    [NCORES * HD, SQ], bf16,
                    tag=f"a2a_out{h}", name=f"a2a_out{h}",
                )
                for h in range(HL)
            ]

            # ---------------- Stage 1: projections + RoPE ----------------
            with (
                tc.tile_pool(name="s1w", bufs=1) as s1w,
                tc.tile_pool(name="s1x", bufs=10) as s1x,
                tc.tile_pool(name="rope", bufs=3) as ropep,
                tc.tile_pool(name="s1v", bufs=3) as s1v,
                tc.tile_pool(name="ps_q", bufs=1, space="PSUM") as ps_q,
                tc.tile_pool(name="ps_kv", bufs=1, space="PSUM") as ps_kv,
                tc.tile_pool(name="ps_tr", bufs=1, space="PSUM") as ps_tr,
                tc.tile_pool(name="ps_sw", bufs=1, space="PSUM") as ps_sw,
            ):
                wq_sb = s1w.tile([128, DC * DL], f32r, tag="wq")
                wk_sb = s1w.tile([128, DC * HD], f32r, tag="wk")
                wv_sb = s1w.tile([128, DC * HD], f32r, tag="wv")
                def load_kv_quarter(cq):
                    nc.sync.dma_start(
                        wk_sb[:, cq * 8 * HD : (cq + 1) * 8 * HD],
                        wkT[cq * 8 * 128 : (cq + 1) * 8 * 128, :].rearrange(
                            "(c p) m -> p c m", p=128
                        ),
                    )
                    nc.sync.dma_start(
                        wv_sb[:, cq * 8 * HD : (cq + 1) * 8 * HD],
                        wvT[cq * 8 * 128 : (cq + 1) * 8 * 128, :].rearrange(
                            "(c p) m -> p c m", p=128
                        ),
                    )

                def load_wq_pair(k):  # 512KB: covers c in {2k, 2k+1}
                    nc.scalar.dma_start(
                        wq_sb[:, k * 2 * DL : (k + 1) * 2 * DL],
                        wqT[k * 2 * 128 : (k + 1) * 2 * 128, :].rearrange(
                            "(c p) m -> p c m", p=128
                        ),
                    )

                load_kv_quarter(0)
                for k in range(3):
                    load_wq_pair(k)

                def rope(dst, stg, col0, ncol):
                    # dst/stg: [128, ncol]; rows 0:64 = even comps, 64:128 odd
                    # dst = stg * cfull + halfswap(stg) * sfull, with
                    # cfull = [c; c] and sfull = [-s; s] staged host-side.
                    swp = ps_sw.tile([128, 512], fp32, tag="swp")
                    nc.tensor.matmul(
                        swp[:, 0:ncol], lhsT=pswr[:], rhs=stg[:, 0:ncol],
                        start=True, stop=True,
                    )
                    t1 = ropep.tile([128, 512], fp32, tag="t1")
                    nc.vector.tensor_tensor(
                        t1[:, 0:ncol], stg[:, 0:ncol].bitcast(fp32),
                        c_sb[:, col0 : col0 + ncol], OP.mult,
                    )
                    t2 = ropep.tile([128, 512], fp32, tag="t2")
                    nc.vector.tensor_tensor(
                        t2[:, 0:ncol], swp[:, 0:ncol],
                        s_sb[:, col0 : col0 + ncol], OP.mult,
                    )
                    nc.vector.tensor_tensor(
                        dst, t1[:, 0:ncol], t2[:, 0:ncol], OP.add
                    )

                for nb in range(NB):
                    qps = [
                        ps_q.tile([128, 512], fp32, tag=f"q{m}", name=f"q{m}")
                        for m in range(HL)
                    ]
                    kps = ps_kv.tile([128, 512], fp32, tag="kk")
                    vps = ps_kv.tile([128, 512], fp32, tag="vv")
                    for c in range(DC):
                        if nb == 0:
                            # stream remaining weight chunks just ahead of
                            # their consumption so they don't head-block xt
                            if c % 2 == 0 and 2 <= c <= 26:
                                load_wq_pair(c // 2 + 2)
                            if c in (6, 14, 22):
                                load_kv_quarter(c // 8 + 1)
                            if c == 3:
                                nc.sync.dma_start(c_sb[:], cosT[:, :])
                                nc.sync.dma_start(s_sb[:], sinT[:, :])
                        xt = s1x.tile([128, 512], f32r, tag="xt")
                        nc.gpsimd.dma_start(xt[:], xT[c, nb, :, :])
                        st = c == 0
                        sp = c == DC - 1
                        for m in range(HL):
                            nc.tensor.matmul(
                                qps[m][:],
                                lhsT=wq_sb[:, c * DL + m * 128 : c * DL + (m + 1) * 128],
                                rhs=xt[:],
                                start=st,
                                stop=sp,
                            )
                        nc.tensor.matmul(
                            kps[:],
                            lhsT=wk_sb[:, c * HD : (c + 1) * HD],
                            rhs=xt[:],
                            start=st,
                            stop=sp,
                        )
                        nc.tensor.matmul(
                            vps[:],
                            lhsT=wv_sb[:, c * HD : (c + 1) * HD],
                            rhs=xt[:],
                            start=st,
                            stop=sp,
                        )
                    # stage PSUM strips to SBUF on ScalarE (fast release of the
                    # accumulating banks), then RoPE on DVE from the copies.
                    # k first: SDPA head 0 is gated on kT completion.
                    stk = s1v.tile([128, 512], f32r, tag="stq")
                    nc.scalar.copy(stk[:], kps[:])
                    rope(kT_sb[:, nb * 512 : (nb + 1) * 512], stk[:], nb * 512, 512)
                    for m in range(HL):
                        stg = s1v.tile([128, 512], f32r, tag="stq")
                        nc.scalar.copy(stg[:], qps[m][:])
                        rope(
                            qT_sb[:, m * S + nb * 512 : m * S + (nb + 1) * 512],
                            stg[:],
                            nb * 512,
                            512,
                        )
                    # vT psum -> sbuf, then PE-transpose each 128-col chunk to
                    # natural [seq, hd] layout with a ones column appended.
                    vt = s1v.tile([128, 512], fp32, tag="vt")
                    nc.scalar.copy(vt[:], vps[:])
                    for j in range(4):
                        kcg = nb * 4 + j
                        vtp = ps_tr.tile([128, 128], fp32, tag="vtr")
                        nc.tensor.transpose(
                            vtp[:], vt[:, j * 128 : (j + 1) * 128], ident[:]
                        )
                        nc.scalar.copy(
                            v_sb[:, kcg * VST : kcg * VST + 128], vtp[:]
                        )
                        nc.vector.memset(
                            v_sb[:, kcg * VST + 128 : kcg * VST + 129], 1.0
                        )

            # ---------------- Stage 2: SDPA per head + AllToAll ----------------
            with (
                tc.tile_pool(name="wo", bufs=11) as wop,
                tc.tile_pool(name="wolh", bufs=1) as wolh,
                tc.tile_pool(name="sd", bufs=2) as sd,
                tc.tile_pool(name="sds", bufs=2) as sds,
                tc.tile_pool(name="msk", bufs=4) as mskp,
            ):
                # Four constant bf16 causal tiles (one per kc%4): -100 where
                # q < k within the diagonal 128x512 block, 0 elsewhere. Added
                # into the logit PSUM via a bf16 identity matmul.
                idb = sd.tile([128, 128], bf16, tag="idb", bufs=1)
                nc.scalar.copy(idb[:], ident[:])
                cmask = sd.tile([128, 4 * 512], bf16, tag="cmask", bufs=1)
                nc.vector.memset(cmask[:], 0.0)
                for j in range(4):
                    if causal:
                        # keep 0 where f - p - 128j >= 0 else fill -100
                        nc.gpsimd.affine_select(
                            out=cmask[:, j * 512 : (j + 1) * 512],
                            in_=cmask[:, j * 512 : (j + 1) * 512],
                            pattern=[[1, 512]],
                            compare_op=OP.is_ge,
                            fill=-100.0,
                            base=-128 * j,
                            channel_multiplier=-1,
                        )
                # per-kc et strip offsets (strip kc starts at column base
                # q_lo1024(kc); packed to skip fully-masked regions)
                et_lo = [
                    ((((kc * 128) // 512) // 2) * 1024 if causal else 0)
                    for kc in range(KC)
                ]
                et_w = [S - lo for lo in et_lo]
                et_off = [sum(et_w[:kc]) for kc in range(KC)]
                et_cols = sum(et_w)
                sdpa_ps = tc.tile_pool(name="ps_l", bufs=2, space="PSUM")
                ps_l = sdpa_ps.__enter__()
                sdpa_ps2 = tc.tile_pool(name="ps_o", bufs=2, space="PSUM")
                ps_o = sdpa_ps2.__enter__()
                for h in range(HL):
                    et = sd.tile(
                        [128, et_cols], bf16, tag="et",
                        bufs=2 if causal else 1,
                    )
                    for kc in range(KC):
                        qb_d = (kc * 128) // 512 if causal else 0
                        q_lo = et_lo[kc]  # pl tiles are 1024 wide
                        for t0 in range(q_lo, S, 1024):
                            width = min(1024, S - t0)
                            pl = ps_l.tile([128, 1024], fp32, tag="pl")
                            diag_here = t0 <= qb_d * 512 < t0 + 1024
                            if causal and diag_here:
                                fo = qb_d * 512 - t0
                                nc.tensor.matmul(
                                    pl[:, fo : fo + 512],
                                    lhsT=idb[:],
                                    rhs=cmask[:, (kc % 4) * 512 : (kc % 4 + 1) * 512],
                                    start=True,
                                    stop=False,
                                )
                            qbs = range(t0 // 512, (t0 + width) // 512)  # 2 slices
                            if (not causal):
                                for qb in qbs:
                                    fo = qb * 512 - t0
                                    mt = mskp.tile([128, 512], fp32, tag="mt")
                                    nc.sync.dma_start(
                                        mt[:],
                                        maskT[
                                            kc * 128 : (kc + 1) * 128,
                                            qb * 512 : (qb + 1) * 512,
                                        ],
                                    )
                                    nc.tensor.matmul(
                                        pl[:, fo : fo + 512],
                                        lhsT=ident[:],
                                        rhs=mt[:],
                                        start=True,
                                        stop=False,
                                    )
                            for qb in qbs:
                                if causal and qb < qb_d:
                                    continue  # fully masked; PSUM junk unread
                                fo = qb * 512 - t0
                                first = not (
                                    (causal and diag_here and qb == qb_d)
                                    or (not causal)
                                )
                                nc.tensor.matmul(
                                    pl[:, fo : fo + 512],
                                    lhsT=kT_sb[:, kc * 128 : (kc + 1) * 128],
                                    rhs=qT_sb[:, h * S + qb * 512 : h * S + (qb + 1) * 512],
                                    start=first,
                                    stop=True,
                                )
                            e0 = max(t0, qb_d * 512) if causal else t0
                            base = et_off[kc] - q_lo
                            nc.scalar.activation(
                                et[:, base + e0 : base + t0 + width],
                                pl[:, e0 - t0 : width],
                                AF.Exp,
                            )
                    # p @ [v | 1] accumulation over k-chunks, per q-chunk
                    for qc in range(KC):
                        kc_hi = qc if causal else KC - 1
                        po = ps_o.tile([128, 129], fp32, tag="po")
                        for kc in range(kc_hi + 1):
                            nc.tensor.matmul(
                                po[:],
                                lhsT=et[
                                    :,
                                    et_off[kc] - et_lo[kc] + qc * 128 : et_off[kc]
                                    - et_lo[kc]
                                    + qc * 128
                                    + 128,
                                ],
                                rhs=v_sb[:, kc * VST : kc * VST + 129],
                                start=(kc == 0),
                                stop=(kc == kc_hi),
                            )
                        rc = sds.tile([128, 1], fp32, tag="rc")
                        nc.vector.reciprocal(rc[:], po[:, 128:129])
                        osb = sds.tile([128, 128], fp32, tag="osb")
                        nc.vector.tensor_scalar_mul(osb[:], po[:, 0:128], rc[:])
                        otp = ps_o.tile([128, 129], fp32, tag="po", name="otp")
                        nc.tensor.transpose(otp[:, 0:128], osb[:], ident[:])
                        if qc % 2 == 0:
                            ots = sds.tile([128, 256], bf16, tag="ots")
                        nc.vector.tensor_copy(
                            ots[:, (qc % 2) * 128 : (qc % 2 + 1) * 128],
                            otp[:, 0:128],
                        )
                        if qc % 2 == 1:
                            nc.sync.dma_start(
                                a2a_in[h][(qc // 2) * 128 : (qc // 2 + 1) * 128, :],
                                ots[:],
                            )
                    if for_sim:
                        # timing proxy: collective replaced by local DMA
                        nc.sync.dma_start(a2a_out[h][:], a2a_in[h][:])
                    else:
                        nc.gpsimd.collective_compute(
                            "AllToAll",
                            OP.bypass,
                            replica_groups=rg,
                            ins=[a2a_in[h][:].opt()],
                            outs=[a2a_out[h][:].opt()],
                        )

                sdpa_ps2.__exit__(None, None, None)
                sdpa_ps.__exit__(None, None, None)
                # ------------- Stage 3: output projection -------------
                with (
                    tc.tile_pool(name="woob", bufs=2) as woob,
                    tc.tile_pool(name="ps_w", bufs=2, space="PSUM") as ps_w,
                ):
                    lh_sb = wolh.tile([128, DC * SQ], bf16, tag="lh")
                    lh4 = lh_sb.rearrange("p (rr hh q) -> p rr hh q", rr=NCORES, hh=HL)
                    for h in range(HL):
                        nc.sync.dma_start(
                            lh4[:, :, h, :],
                            a2a_out[h].rearrange("(rr p) q -> p rr q", p=128),
                        )
                    # accumulate din chunks grouped by head so the first 3/4 of
                    # each PSUM accumulation can run while later heads are still
                    # in SDPA (engines are in-order; emission order matters)
                    corder = [rr * HL + h for h in range(HL) for rr in range(NCORES)]
                    for nbog in range(4):  # 2 dout blocks of 512 per group
                        pw = [
                            ps_w.tile([128, 512], fp32, tag=f"wo{m}", name=f"pw{m}")
                            for m in range(4)
                        ]
                        for ci, c in enumerate(corder):
                            wt = wop.tile([128, 1024], bf16, tag="wt")
                            dma_eng = nc.gpsimd if ci % 2 == 0 else nc.scalar
                            dma_eng.dma_start(wt[:], woT[c, nbog, :, :])
                            for m in range(4):
                                nc.tensor.matmul(
                                    pw[m][:],
                                    lhsT=lh_sb[:, c * SQ + (m % 2) * 128 : c * SQ + (m % 2 + 1) * 128],
                                    rhs=wt[:, (m // 2) * 512 : (m // 2 + 1) * 512],
                                    start=(ci == 0),
                                    stop=(ci == DC - 1),
                                )
                        for m in range(4):
                            ob = woob.tile([128, 512], fp32, tag="ob")
                            nc.vector.tensor_copy(ob[:], pw[m][:])
                            nc.sync.dma_start(
                                out[
                                    (m % 2) * 128 : (m % 2 + 1) * 128,
                                    (nbog * 2 + m // 2) * 512 : (nbog * 2 + m // 2 + 1) * 512,
                                ],
                                ob[:],
                            )
    nc.compile()
    return nc


_PERM = np.concatenate([np.arange(0, HD, 2), np.arange(1, HD, 2)])


def _stage_inputs(x, wq, wk, wv, wo, mask, freqs_cos, freqs_sin, causal):
    alpha = float(HD) ** -0.25  # sqrt of logit scale folded into both ropes
    import ml_dtypes

    xTc = np.ascontiguousarray(
        x.T.reshape(DC, 128, NB, 512).transpose(0, 2, 1, 3)
    )
    woTc = np.ascontiguousarray(
        wo.T.reshape(DC, 128, 4, 1024).transpose(0, 2, 1, 3)
    ).astype(ml_dtypes.bfloat16)
    ct = freqs_cos.T * alpha
    st = freqs_sin.T * alpha
    cosTc = np.ascontiguousarray(np.concatenate([ct, ct], axis=0))
    sinTc = np.ascontiguousarray(np.concatenate([-st, st], axis=0))
    if not causal:
        maskTc = np.ascontiguousarray(np.maximum(mask, -60.0).T)
    in_maps = []
    for i in range(NCORES):
        wq_i = wq[i * DL : (i + 1) * DL, :].reshape(HL, HD, D)[:, _PERM, :]
        wk_i = wk[i * HD : (i + 1) * HD, :][_PERM, :]
        wv_i = wv[i * HD : (i + 1) * HD, :]
        m = dict(
            xT=xTc,
            wqT=np.ascontiguousarray(wq_i.reshape(DL, D).T),
            wkT=np.ascontiguousarray(wk_i.T),
            wvT=np.ascontiguousarray(wv_i.T),
            cosT=cosTc,
            sinT=sinTc,
            woT=woTc,
        )
        if not causal:
            m["maskT"] = maskTc
        in_maps.append(m)
    return in_maps


def _is_causal(mask):
    if mask.shape != (S, S):
        return False
    tri = np.tril(np.ones((S, S), bool))
    return bool(
        np.all(mask[tri] == 0.0) and np.all(mask[~tri] <= -1e8)
    )


def run(inputs, trace=False):
    from concourse.bass_utils import run_bass_kernel_spmd

    causal = _is_causal(np.asarray(inputs["mask"]))
    if causal not in _built:
        _built[causal] = _build(causal)
    nc = _built[causal]
    in_maps = _stage_inputs(
        np.asarray(inputs["x"], np.float32),
        np.asarray(inputs["wq"], np.float32),
        np.asarray(inputs["wk"], np.float32),
        np.asarray(inputs["wv"], np.float32),
        np.asarray(inputs["wo"], np.float32),
        np.asarray(inputs["mask"], np.float32),
        np.asarray(inputs["freqs_cos"], np.float32),
        np.asarray(inputs["freqs_sin"], np.float32),
        causal,
    )
    res = run_bass_kernel_spmd(
        nc, in_maps, core_ids=list(range(NCORES)), trace=trace
    )
    out = np.concatenate([res.results[i]["out"] for i in range(NCORES)], axis=0)
    return out, res


def kernel(**inputs):
    out, _ = run(inputs, trace=False)
    return out

